# revision 11
# baseline (speedup 1.0000x reference)
"""Trainium2 Bass kernel for nn_Att_mlp_softmax (GNN message passing).

Reference computation:
    e = relu(h @ W1 + b1) @ W2 + b2                       # [N, 1] per-node score
    att = softmax(where(G > 0, e.T broadcast, -9e15))     # row-wise over neighbors
    out = (G.sum(-1))[:, None] * (att @ h)                # degree-rescaled aggregation

Because the pre-softmax score of entry (i, j) depends only on column j, the
masked softmax collapses algebraically:
    att[i, j] = G[i, j] * w[j] / sum_j G[i, j] * w[j],  w = exp(e + ESHIFT)
so with H' = [w * h | w | 1] (N x 130):
    Y = G @ H'
    out = Y[:, 129] * Y[:, :128] / Y[:, 128]
One big [N, N] x [N, 130] matmul replaces the N^2 softmax entirely.

Precision/perf: the harness gate is rel_err < 2e-2; a full single-bf16 pipeline
measures ~4e-3 in numpy emulation, so no hi/lo splitting anywhere.  G is an
exact 0/1 mask streamed as fp8e4 (quarter the fp32 HBM traffic) and used
directly as the matmul stationary operand against bf16 moving data (mixed
dtypes are legal; cost keys on the moving dtype).  The steady-state main loop
issues one 130-column matmul per 57 ns with LDWEIGHTS fully hidden.

The MLP score e is computed WITHOUT the tensor engine beyond the z matmuls:
e = sum_hid(W2 (.) relu-out) via DVE 32x32 stream-transposes (8 batched ops
per quarter) + a free-axis reduce.  Per-chunk-stationary e matmuls would cost
~10 us of unhidden LDWEIGHTS on the PE.

Distribution: G is row-sharded across 8 NeuronCores (1024 rows each); h and
the MLP weights are replicated.  Each core's G shard is laid out
[128, JC, ROWS] (contraction-position major) so every DMA line is 8 KB
contiguous.  h is passed twice in bf16: d-major (hT, the MLP moving operand)
and chunk-major (hc, for the H' build).  The output is stored p-major
([128, 8, D], contiguous 4 KB partition lines; host inverts) because the
natural layout fragments the final DMA into 512 B packets.  No collectives.
"""

import numpy as np

N = 8192
D = 128
HID = 64
N_CORES = 8
ROWS = N // N_CORES          # 1024 output rows per core
JC = N // 128                # 64 contraction chunks of 128
GRP = 8                      # contraction chunks per G DMA (1 MB transfers)
NCOL = 130                   # H' columns: [w*h | w | 1]
ESHIFT = -1.0                # exp(e - 1): cancels exactly in the ratio
WARM = 4                     # PE clock-ramp dummy matmuls

_cache = {}


def _install_axon_hooks_shim():
    """Provide antenv.axon_hooks if the image lacks it (trn_boot step 6).

    concourse.bass_utils imports it unconditionally when BASS_TRACE is set;
    without the shim that import crashes instead of degrading.
    """
    import contextlib
    import ctypes
    import sys
    import types

    try:
        import antenv.axon_hooks  # noqa: F401
        return
    except ImportError:
        pass

    so_path = "/opt/axon/libaxon_pjrt.so"

    def _make_hook():
        try:
            lib = ctypes.CDLL(so_path)
        except OSError:
            return None
        if not hasattr(lib, "axon_start_nrt_profile"):
            return None
        lib.axon_start_nrt_profile.argtypes = [
            ctypes.POINTER(ctypes.c_int64),
            ctypes.c_size_t,
        ]
        lib.axon_start_nrt_profile.restype = ctypes.c_int64
        lib.axon_stop_nrt_profile.argtypes = [ctypes.c_char_p]
        lib.axon_stop_nrt_profile.restype = ctypes.c_int64

        @contextlib.contextmanager
        def _hook(output_dir, device_ids):
            import jax

            jax.devices()
            if device_ids:
                ids = (ctypes.c_int64 * len(device_ids))(*device_ids)
                rc = lib.axon_start_nrt_profile(ids, len(device_ids))
            else:
                rc = lib.axon_start_nrt_profile(None, 0)
            if rc != 0:
                raise RuntimeError(f"axon_start_nrt_profile rc={rc}")
            try:
                yield
            finally:
                lib.axon_stop_nrt_profile(str(output_dir).encode())

        return _hook

    mod = types.ModuleType("antenv.axon_hooks")
    _holder = {"hook": _make_hook()}
    mod.set_axon_ntff_profile_hook = lambda h: _holder.__setitem__("hook", h)
    mod.get_axon_ntff_profile_hook = lambda: _holder["hook"]
    sys.modules["antenv.axon_hooks"] = mod
    try:
        import antenv

        antenv.axon_hooks = mod
    except ImportError:
        pass


def build_nc(enable_asserts=False):
    """Build + compile the per-core Bass program (identical on all 8 cores)."""
    from concourse import bacc, mybir, tile

    f32 = mybir.dt.float32
    bf16 = mybir.dt.bfloat16
    f8 = mybir.dt.float8e4
    AF = mybir.ActivationFunctionType
    ALU = mybir.AluOpType
    AX = mybir.AxisListType

    nc = bacc.Bacc(
        "TRN2",
        target_bir_lowering=False,
        debug=False,
        enable_asserts=enable_asserts,
        num_devices=N_CORES,
    )
    g8 = nc.dram_tensor("g8", [128, JC, ROWS], f8, kind="ExternalInput").ap()
    hT = nc.dram_tensor("hT", [D, N], bf16, kind="ExternalInput").ap()
    hc = nc.dram_tensor("hc", [128, JC, D], bf16, kind="ExternalInput").ap()
    W1 = nc.dram_tensor("W1", [D, HID], bf16, kind="ExternalInput").ap()
    b1 = nc.dram_tensor("b1", [HID, 1], f32, kind="ExternalInput").ap()
    W2 = nc.dram_tensor("W2", [HID, 1], f32, kind="ExternalInput").ap()
    # b2e = b2 + ESHIFT pre-broadcast to [128, 1] on the host (exp bias)
    b2e = nc.dram_tensor("b2e", [128, 1], f32, kind="ExternalInput").ap()
    out = nc.dram_tensor("out", [128, 8, D], f32, kind="ExternalOutput").ap()

    with tile.TileContext(nc) as tc:
        with (
            tc.tile_pool(name="const", bufs=1) as cpool,
            tc.tile_pool(name="big", bufs=1) as bigpool,
            tc.tile_pool(name="gbuf", bufs=6) as gpool,
            tc.tile_pool(name="hpbuf", bufs=16) as hpool,
            tc.tile_pool(name="outbuf", bufs=3) as opool,
            tc.tile_pool(name="small", bufs=2) as spool,
        ):
            # DMA-issue instructions cost ~700 ns of issuing-engine time each,
            # so the critical-path transfers lead each queue: consts + hT
            # chunk 0 on sync ahead of the G stream, hT chunk 1 on the scalar
            # engine's HW queue, hc behind on gpsimd (SWDGE).
            W1_sb = cpool.tile([D, HID], bf16)
            nc.sync.dma_start(W1_sb[:], W1[:])
            b1_sb = cpool.tile([HID, 1], f32)
            nc.sync.dma_start(b1_sb[:], b1[:])
            hT_sb = bigpool.tile([D, N], bf16)
            nc.sync.dma_start(hT_sb[:, 0 : N // 2], hT[:, 0 : N // 2])
            nc.scalar.dma_start(hT_sb[:, N // 2 :], hT[:, N // 2 :])
            W2_sb = cpool.tile([HID, 1], f32)
            nc.scalar.dma_start(W2_sb[:], W2[:])
            b2e_sb = cpool.tile([128, 1], f32)
            nc.scalar.dma_start(b2e_sb[:], b2e[:])
            hc_sb = bigpool.tile([128, JC, D], bf16)
            for q in range(4):
                cl = slice(q * (JC // 4), (q + 1) * (JC // 4))
                nc.gpsimd.dma_start(hc_sb[:, cl, :], hc[:, cl, :])

            a_sb = bigpool.tile([HID, N], bf16)    # relu, then (.)W2 in place
            a2T = bigpool.tile([128, JC, HID], bf16)  # a_sb chunk-transposed
            e3 = bigpool.tile([128, JC, 1], f32)   # per-node score e
            w_sb = cpool.tile([128, JC], f32)      # exp(e + ESHIFT)
            wtail = cpool.tile([128, 2, JC], bf16)  # H' tail cols [w, 1]
            nc.vector.memset(wtail[:, 1, :], 1.0)

            with tc.tile_pool(name="ps_pre", bufs=2, space="PSUM") as ps_pre:
                # dummy matmuls on a zero tile trip the PE HAM activity
                # monitor out of its cold clock while hT streams in (warm
                # shares the pz slots, keeping ps_pre at 2 PSUM banks)
                warm = cpool.tile([128, 512], bf16)
                nc.vector.memset(warm[:], 0.0)
                for _ in range(WARM):
                    pw = ps_pre.tile([HID, 512], f32, tag="pz", name="pw")
                    nc.tensor.matmul(
                        pw[:], warm[:, 0:HID], warm[:], start=True, stop=True
                    )

                # z = hT.T @ W1, relu'd into a_sb per 512-block
                for nb in range(N // 512):
                    pz = ps_pre.tile([HID, 512], f32, tag="pz")
                    sl = slice(nb * 512, (nb + 1) * 512)
                    nc.tensor.matmul(
                        pz[:], W1_sb[:], hT_sb[:, sl], start=True, stop=True
                    )
                    nc.scalar.activation(
                        a_sb[:, sl], pz[:], AF.Relu, bias=b1_sb[:]
                    )

                # e entirely on DVE, per quarter of 16 chunks:
                #   a2 = W2 (.) a  (in place)
                #   a2T[p, c, hid] = a2[hid, 128c + p] via 8 batched 32x32
                #     stream-transposes (block (i,j) pairs across 16 chunks)
                #   e = reduce_add(a2T, axis hid); w = exp(e + b2 + ESHIFT)
                QW = JC // 4
                for q in range(4):
                    ql = slice(q * QW, (q + 1) * QW)
                    nsl = slice(q * QW * 128, (q + 1) * QW * 128)
                    nc.vector.tensor_scalar_mul(
                        a_sb[:, nsl], a_sb[:, nsl], W2_sb[:]
                    )
                    ar = a_sb.rearrange("h (c v) -> h c v", v=128)
                    for i in range(HID // 32):
                        for j in range(128 // 32):
                            nc.vector.transpose(
                                a2T[
                                    32 * j : 32 * (j + 1),
                                    ql,
                                    32 * i : 32 * (i + 1),
                                ],
                                ar[
                                    32 * i : 32 * (i + 1),
                                    ql,
                                    32 * j : 32 * (j + 1),
                                ],
                            )
                    nc.vector.tensor_reduce(
                        e3[:, ql, :], a2T[:, ql, :], AX.X, ALU.add
                    )
                    nc.scalar.activation(
                        w_sb[:, ql], e3[:, ql, 0], AF.Exp, bias=b2e_sb[:]
                    )
                    nc.vector.tensor_copy(wtail[:, 0, ql], w_sb[:, ql])

            # Main accumulation: acc[it] [128, NCOL] += G_tile.T @ H'_chunk.
            with tc.tile_pool(name="ps_acc", bufs=8, space="PSUM") as ps_acc:
                accs = [
                    ps_acc.tile([128, NCOL], f32, tag="acc", name=f"acc{i}")
                    for i in range(8)
                ]

                def build_hp(jc):
                    # just-in-time H' chunk build: 2 DVE ops
                    hp = hpool.tile([128, NCOL], bf16, tag="hp",
                                    name=f"hp{jc}")
                    nc.vector.tensor_scalar_mul(
                        hp[:, 0:128], hc_sb[:, jc, :], w_sb[:, jc : jc + 1]
                    )
                    nc.vector.tensor_copy(hp[:, 128:130], wtail[:, :, jc])
                    return hp

                for jg in range(JC // GRP - 1):
                    gt = gpool.tile([128, GRP, ROWS], f8, tag="gt")
                    nc.sync.dma_start(
                        gt[:], g8[:, jg * GRP : (jg + 1) * GRP, :]
                    )
                    for jci in range(GRP):
                        jc = jg * GRP + jci
                        hp = build_hp(jc)
                        for it in range(8):
                            nc.tensor.matmul(
                                accs[it][:],
                                gt[:, jci, it * 128 : (it + 1) * 128],
                                hp[:],
                                start=(jc == 0),
                                stop=False,
                            )

                # last group runs it-major with staggered stop so each bank's
                # epilogue overlaps the remaining banks' matmuls
                gt = gpool.tile([128, GRP, ROWS], f8, tag="gt", name="gt_last")
                nc.sync.dma_start(gt[:], g8[:, JC - GRP :, :])
                hps_last = [build_hp(JC - GRP + jci) for jci in range(GRP)]
                for it in range(8):
                    for jci in range(GRP):
                        nc.tensor.matmul(
                            accs[it][:],
                            gt[:, jci, it * 128 : (it + 1) * 128],
                            hps_last[jci][:],
                            start=False,
                            stop=(jci == GRP - 1),
                        )

                # epilogue, fully per-bank: each bank's whole chain (tail
                # copy -> recip -> r -> scaled output) runs as soon as ITS
                # accumulator stops; only bank 7's short chain + the out DMA
                # trail the loop. (one PSUM operand per DVE op)
                ot_all = opool.tile([128, 8, D], f32, tag="ot_all", bufs=1)
                for it in range(8):
                    tl = spool.tile([128, 2], f32, tag="tl", name=f"tl{it}",
                                    bufs=8)
                    nc.vector.tensor_copy(tl[:], accs[it][:, 128:130])
                    den = spool.tile([128, 1], f32, tag="den",
                                     name=f"den{it}", bufs=8)
                    nc.vector.tensor_scalar_add(den[:], tl[:, 0:1], 1e-30)
                    rc = spool.tile([128, 1], f32, tag="rc", name=f"rc{it}",
                                    bufs=8)
                    nc.vector.reciprocal(rc[:], den[:])
                    r1 = spool.tile([128, 1], f32, tag="r1", name=f"r1{it}",
                                    bufs=8)
                    nc.vector.tensor_mul(r1[:], rc[:], tl[:, 1:2])
                    nc.vector.tensor_scalar_mul(
                        ot_all[:, it, :], accs[it][:, 0:128], r1[:]
                    )
                nc.sync.dma_start(out[:], ot_all[:])

    nc.compile()
    return nc


def make_in_maps(graph_info, h, W1, b1, W2, b2):
    """Shard + lay out the full inputs for the 8 cores."""
    import ml_dtypes

    bf16 = ml_dtypes.bfloat16
    f8 = ml_dtypes.float8_e4m3fn

    # G (exact 0/1) as fp8, laid out [core][128 c, JC, ROWS] so the stationary
    # tile for (chunk jc, row block it) is g8[:, jc, it*128:(it+1)*128] and
    # every per-partition DMA line is contiguous
    g = np.asarray(graph_info, np.float32)
    G8 = g.astype(f8).reshape(N_CORES, ROWS, JC, 128).transpose(0, 3, 2, 1)
    h = np.asarray(h, np.float32)
    hTb = np.ascontiguousarray(h.T).astype(bf16)               # [D, N]
    hcb = np.ascontiguousarray(
        h.reshape(JC, 128, D).transpose(1, 0, 2)               # [128, JC, D]
    ).astype(bf16)
    W1b = np.asarray(W1, np.float32).astype(bf16)
    b1r = np.asarray(b1, np.float32).reshape(HID, 1)
    W2r = np.asarray(W2, np.float32).reshape(HID, 1)
    b2e = np.full((128, 1), float(np.asarray(b2).reshape(())) + ESHIFT,
                  np.float32)
    in_maps = []
    for c in range(N_CORES):
        in_maps.append(
            {
                "g8": np.ascontiguousarray(G8[c]),
                "hT": hTb,
                "hc": hcb,
                "W1": W1b,
                "b1": b1r,
                "W2": W2r,
                "b2e": b2e,
            }
        )
    return in_maps


def kernel(graph_info, h, W1, b1, W2, b2):
    _install_axon_hooks_shim()
    from concourse.bass_utils import run_bass_kernel_spmd

    if "nc" not in _cache:
        _cache["nc"] = build_nc()
    nc = _cache["nc"]

    in_maps = make_in_maps(graph_info, h, W1, b1, W2, b2)
    res = run_bass_kernel_spmd(nc, in_maps, list(range(N_CORES)))
    # out is stored p-major [128, 8, D] per core; invert to row order
    return np.concatenate(
        [
            res.results[c]["out"].transpose(1, 0, 2).reshape(ROWS, D)
            for c in range(N_CORES)
        ],
        axis=0,
    )


# revision 17
# speedup vs baseline: 1.2676x; 1.2676x over previous
"""Trainium2 Bass kernel for nn_Att_mlp_softmax (GNN message passing).

Reference computation:
    e = relu(h @ W1 + b1) @ W2 + b2                       # [N, 1] per-node score
    att = softmax(where(G > 0, e.T broadcast, -9e15))     # row-wise over neighbors
    out = (G.sum(-1))[:, None] * (att @ h)                # degree-rescaled aggregation

Because the pre-softmax score of entry (i, j) depends only on column j, the
masked softmax collapses algebraically:
    att[i, j] = G[i, j] * w[j] / sum_j G[i, j] * w[j],  w = exp(e + ESHIFT)
so with H' = [w * h | w | 1] (N x 130):
    Y = G @ H'
    out = Y[:, 129] * Y[:, :128] / Y[:, 128]
One big [N, N] x [N, 130] matmul replaces the N^2 softmax entirely.

Precision/perf: the harness gate is rel_err < 2e-2; a full single-bf16 pipeline
measures ~4e-3 in numpy emulation, so no hi/lo splitting anywhere.  G is an
exact 0/1 mask streamed as fp8e4 (quarter the fp32 HBM traffic) and used
directly as the matmul stationary operand against bf16 moving data (mixed
dtypes are legal; cost keys on the moving dtype).  The steady-state main loop
issues one 130-column matmul per 57 ns with LDWEIGHTS fully hidden.

The MLP z/e phase is pipelined per quarter (z blocks -> relu -> 16 e matmuls
-> exp -> w tails) so the first quarter of w unblocks the main loop early.
(A DVE stream-transpose e variant was measured SLOWER: 700 ns per 32-partition
transpose op vs ~7 us total for the PE e matmuls.)

Distribution: G is row-sharded across 8 NeuronCores (1024 rows each); h and
the MLP weights are replicated.  Each core's G shard is laid out
[128, JC, ROWS] (contraction-position major) so every DMA line is 8 KB
contiguous.  h is passed twice in bf16: d-major (hT, the MLP moving operand)
and chunk-major (hc, for the H' build).  The output is stored p-major
([128, 8, D], contiguous 4 KB partition lines; host inverts) because the
natural layout fragments the final DMA into 512 B packets.  No collectives.
"""

import numpy as np

N = 8192
D = 128
HID = 64
N_CORES = 8
ROWS = N // N_CORES          # 1024 output rows per core
JC = N // 128                # 64 contraction chunks of 128
GRP = 8                      # contraction chunks per G DMA (1 MB transfers)
NCOL = 130                   # H' columns: [w*h | w | 1]
ESHIFT = -1.0                # exp(e - 1): cancels exactly in the ratio
WARM = 4                     # PE clock-ramp dummy matmuls

_cache = {}


def _install_axon_hooks_shim():
    """Provide antenv.axon_hooks if the image lacks it (trn_boot step 6).

    concourse.bass_utils imports it unconditionally when BASS_TRACE is set;
    without the shim that import crashes instead of degrading.
    """
    import contextlib
    import ctypes
    import sys
    import types

    try:
        import antenv.axon_hooks  # noqa: F401
        return
    except ImportError:
        pass

    so_path = "/opt/axon/libaxon_pjrt.so"

    def _make_hook():
        try:
            lib = ctypes.CDLL(so_path)
        except OSError:
            return None
        if not hasattr(lib, "axon_start_nrt_profile"):
            return None
        lib.axon_start_nrt_profile.argtypes = [
            ctypes.POINTER(ctypes.c_int64),
            ctypes.c_size_t,
        ]
        lib.axon_start_nrt_profile.restype = ctypes.c_int64
        lib.axon_stop_nrt_profile.argtypes = [ctypes.c_char_p]
        lib.axon_stop_nrt_profile.restype = ctypes.c_int64

        @contextlib.contextmanager
        def _hook(output_dir, device_ids):
            import jax

            jax.devices()
            if device_ids:
                ids = (ctypes.c_int64 * len(device_ids))(*device_ids)
                rc = lib.axon_start_nrt_profile(ids, len(device_ids))
            else:
                rc = lib.axon_start_nrt_profile(None, 0)
            if rc != 0:
                raise RuntimeError(f"axon_start_nrt_profile rc={rc}")
            try:
                yield
            finally:
                lib.axon_stop_nrt_profile(str(output_dir).encode())

        return _hook

    mod = types.ModuleType("antenv.axon_hooks")
    _holder = {"hook": _make_hook()}
    mod.set_axon_ntff_profile_hook = lambda h: _holder.__setitem__("hook", h)
    mod.get_axon_ntff_profile_hook = lambda: _holder["hook"]
    sys.modules["antenv.axon_hooks"] = mod
    try:
        import antenv

        antenv.axon_hooks = mod
    except ImportError:
        pass


def build_nc(enable_asserts=False):
    """Build + compile the per-core Bass program (identical on all 8 cores)."""
    from concourse import bacc, mybir, tile

    f32 = mybir.dt.float32
    bf16 = mybir.dt.bfloat16
    f8 = mybir.dt.float8e4
    AF = mybir.ActivationFunctionType
    ALU = mybir.AluOpType

    nc = bacc.Bacc(
        "TRN2",
        target_bir_lowering=False,
        debug=False,
        enable_asserts=enable_asserts,
        num_devices=N_CORES,
    )
    g8 = nc.dram_tensor("g8", [128, JC, ROWS], f8, kind="ExternalInput").ap()
    hT = nc.dram_tensor("hT", [D, N], bf16, kind="ExternalInput").ap()
    hc = nc.dram_tensor("hc", [128, JC, D], bf16, kind="ExternalInput").ap()
    W1 = nc.dram_tensor("W1", [D, HID], bf16, kind="ExternalInput").ap()
    b1 = nc.dram_tensor("b1", [HID, 1], f32, kind="ExternalInput").ap()
    W2 = nc.dram_tensor("W2", [HID, 1], bf16, kind="ExternalInput").ap()
    # b2e = b2 + ESHIFT pre-broadcast to [128, 1] on the host (exp bias)
    b2e = nc.dram_tensor("b2e", [128, 1], f32, kind="ExternalInput").ap()
    out = nc.dram_tensor("out", [128, 8, D], f32, kind="ExternalOutput").ap()

    with tile.TileContext(nc) as tc:
        with (
            tc.tile_pool(name="const", bufs=1) as cpool,
            tc.tile_pool(name="big", bufs=1) as bigpool,
            tc.tile_pool(name="gbuf", bufs=6) as gpool,
            tc.tile_pool(name="hpbuf", bufs=16) as hpool,
            tc.tile_pool(name="outbuf", bufs=3) as opool,
            tc.tile_pool(name="small", bufs=2) as spool,
        ):
            # DMA-issue instructions cost ~700 ns of issuing-engine time each,
            # so the critical-path transfers lead each queue: consts + hT
            # chunk 0 on sync ahead of the G stream, hT chunk 1 on the scalar
            # engine's HW queue, hc behind on gpsimd (SWDGE).
            W1_sb = cpool.tile([D, HID], bf16)
            nc.sync.dma_start(W1_sb[:], W1[:])
            b1_sb = cpool.tile([HID, 1], f32)
            nc.sync.dma_start(b1_sb[:], b1[:])
            hT_sb = bigpool.tile([D, N], bf16)
            nc.sync.dma_start(hT_sb[:, 0 : N // 2], hT[:, 0 : N // 2])
            nc.scalar.dma_start(hT_sb[:, N // 2 :], hT[:, N // 2 :])
            W2_sb = cpool.tile([HID, 1], bf16)
            nc.scalar.dma_start(W2_sb[:], W2[:])
            b2e_sb = cpool.tile([128, 1], f32)
            nc.scalar.dma_start(b2e_sb[:], b2e[:])
            hc_sb = bigpool.tile([128, JC, D], bf16)
            for q in range(4):
                cl = slice(q * (JC // 4), (q + 1) * (JC // 4))
                nc.gpsimd.dma_start(hc_sb[:, cl, :], hc[:, cl, :])

            a_sb = bigpool.tile([HID, N], bf16)    # relu(h @ W1 + b1)
            w_sb = cpool.tile([128, JC], f32)      # exp(e + ESHIFT)
            wtail = cpool.tile([128, 2, JC], bf16)  # H' tail cols [w, 1]
            nc.vector.memset(wtail[:, 1, :], 1.0)

            with tc.tile_pool(name="ps_pre", bufs=2, space="PSUM") as ps_pre:
                # dummy matmuls on a zero tile trip the PE HAM activity
                # monitor out of its cold clock while hT streams in (warm
                # shares the pz slots, keeping ps_pre at 3 PSUM banks)
                warm = cpool.tile([128, 512], bf16)
                nc.vector.memset(warm[:], 0.0)
                for _ in range(WARM):
                    pw = ps_pre.tile([HID, 512], f32, tag="pz", name="pw")
                    nc.tensor.matmul(
                        pw[:], warm[:, 0:HID], warm[:], start=True, stop=True
                    )

                # MLP pipeline per quarter: z (4 blocks of 512) -> relu ->
                # e (16 chunk-stationary matmuls against W2) -> exp -> w
                # tails.  The first quarter unblocks the main loop while
                # later scores still compute.
                pe = ps_pre.tile([128, JC], f32, tag="pe", bufs=1)
                QW = JC // 4
                for q in range(4):
                    for nb in range(4 * q, 4 * (q + 1)):
                        pz = ps_pre.tile([HID, 512], f32, tag="pz")
                        sl = slice(nb * 512, (nb + 1) * 512)
                        nc.tensor.matmul(
                            pz[:], W1_sb[:], hT_sb[:, sl], start=True,
                            stop=True,
                        )
                        nc.scalar.activation(
                            a_sb[:, sl], pz[:], AF.Relu, bias=b1_sb[:]
                        )
                    for c in range(q * QW, (q + 1) * QW):
                        nc.tensor.matmul(
                            pe[:, c : c + 1],
                            a_sb[:, c * 128 : (c + 1) * 128],
                            W2_sb[:],
                            start=True,
                            stop=True,
                        )
                    ql = slice(q * QW, (q + 1) * QW)
                    nc.scalar.activation(
                        w_sb[:, ql], pe[:, ql], AF.Exp, bias=b2e_sb[:]
                    )
                    nc.vector.tensor_copy(wtail[:, 0, ql], w_sb[:, ql])

            # Main accumulation: acc[it] [128, NCOL] += G_tile.T @ H'_chunk.
            with tc.tile_pool(name="ps_acc", bufs=8, space="PSUM") as ps_acc:
                accs = [
                    ps_acc.tile([128, NCOL], f32, tag="acc", name=f"acc{i}")
                    for i in range(8)
                ]

                def build_hp(jc):
                    # just-in-time H' chunk build: 2 DVE ops
                    hp = hpool.tile([128, NCOL], bf16, tag="hp",
                                    name=f"hp{jc}")
                    nc.vector.tensor_scalar_mul(
                        hp[:, 0:128], hc_sb[:, jc, :], w_sb[:, jc : jc + 1]
                    )
                    nc.vector.tensor_copy(hp[:, 128:130], wtail[:, :, jc])
                    return hp

                for jg in range(JC // GRP - 1):
                    gt = gpool.tile([128, GRP, ROWS], f8, tag="gt")
                    nc.sync.dma_start(
                        gt[:], g8[:, jg * GRP : (jg + 1) * GRP, :]
                    )
                    for jci in range(GRP):
                        jc = jg * GRP + jci
                        hp = build_hp(jc)
                        for it in range(8):
                            nc.tensor.matmul(
                                accs[it][:],
                                gt[:, jci, it * 128 : (it + 1) * 128],
                                hp[:],
                                start=(jc == 0),
                                stop=False,
                            )

                # last group runs it-major with staggered stop so each bank's
                # epilogue overlaps the remaining banks' matmuls
                gt = gpool.tile([128, GRP, ROWS], f8, tag="gt", name="gt_last")
                nc.sync.dma_start(gt[:], g8[:, JC - GRP :, :])
                hps_last = [build_hp(JC - GRP + jci) for jci in range(GRP)]
                for it in range(8):
                    for jci in range(GRP):
                        nc.tensor.matmul(
                            accs[it][:],
                            gt[:, jci, it * 128 : (it + 1) * 128],
                            hps_last[jci][:],
                            start=False,
                            stop=(jci == GRP - 1),
                        )

                # epilogue, fully per-bank: each bank's whole chain (tail
                # copy -> recip -> r -> scaled output) runs as soon as ITS
                # accumulator stops; only bank 7's short chain + the out DMA
                # trail the loop. (one PSUM operand per DVE op)
                ot_all = opool.tile([128, 8, D], f32, tag="ot_all", bufs=1)
                for it in range(8):
                    tl = spool.tile([128, 2], f32, tag="tl", name=f"tl{it}",
                                    bufs=8)
                    nc.vector.tensor_copy(tl[:], accs[it][:, 128:130])
                    den = spool.tile([128, 1], f32, tag="den",
                                     name=f"den{it}", bufs=8)
                    nc.vector.tensor_scalar_add(den[:], tl[:, 0:1], 1e-30)
                    rc = spool.tile([128, 1], f32, tag="rc", name=f"rc{it}",
                                    bufs=8)
                    nc.vector.reciprocal(rc[:], den[:])
                    r1 = spool.tile([128, 1], f32, tag="r1", name=f"r1{it}",
                                    bufs=8)
                    nc.vector.tensor_mul(r1[:], rc[:], tl[:, 1:2])
                    nc.vector.tensor_scalar_mul(
                        ot_all[:, it, :], accs[it][:, 0:128], r1[:]
                    )
                nc.sync.dma_start(out[:], ot_all[:])

    nc.compile()
    return nc


def make_in_maps(graph_info, h, W1, b1, W2, b2):
    """Shard + lay out the full inputs for the 8 cores."""
    import ml_dtypes

    bf16 = ml_dtypes.bfloat16
    f8 = ml_dtypes.float8_e4m3fn

    # G (exact 0/1) as fp8, laid out [core][128 c, JC, ROWS] so the stationary
    # tile for (chunk jc, row block it) is g8[:, jc, it*128:(it+1)*128] and
    # every per-partition DMA line is contiguous
    g = np.asarray(graph_info, np.float32)
    G8 = g.astype(f8).reshape(N_CORES, ROWS, JC, 128).transpose(0, 3, 2, 1)
    h = np.asarray(h, np.float32)
    hTb = np.ascontiguousarray(h.T).astype(bf16)               # [D, N]
    hcb = np.ascontiguousarray(
        h.reshape(JC, 128, D).transpose(1, 0, 2)               # [128, JC, D]
    ).astype(bf16)
    W1b = np.asarray(W1, np.float32).astype(bf16)
    b1r = np.asarray(b1, np.float32).reshape(HID, 1)
    W2r = np.asarray(W2, np.float32).reshape(HID, 1).astype(bf16)
    b2e = np.full((128, 1), float(np.asarray(b2).reshape(())) + ESHIFT,
                  np.float32)
    in_maps = []
    for c in range(N_CORES):
        in_maps.append(
            {
                "g8": np.ascontiguousarray(G8[c]),
                "hT": hTb,
                "hc": hcb,
                "W1": W1b,
                "b1": b1r,
                "W2": W2r,
                "b2e": b2e,
            }
        )
    return in_maps


def kernel(graph_info, h, W1, b1, W2, b2):
    _install_axon_hooks_shim()
    from concourse.bass_utils import run_bass_kernel_spmd

    if "nc" not in _cache:
        _cache["nc"] = build_nc()
    nc = _cache["nc"]

    in_maps = make_in_maps(graph_info, h, W1, b1, W2, b2)
    res = run_bass_kernel_spmd(nc, in_maps, list(range(N_CORES)))
    # out is stored p-major [128, 8, D] per core; invert to row order
    return np.concatenate(
        [
            res.results[c]["out"].transpose(1, 0, 2).reshape(ROWS, D)
            for c in range(N_CORES)
        ],
        axis=0,
    )
